# revision 1
# baseline (speedup 1.0000x reference)
"""Trainium2 Bass kernel for nn_MultiHeadAttention_73607149519012.

MHA: B=8, S=1024, D=1024, H=16 heads, depth=64, fp32 in/out.
Sharding: data-parallel over batch -- one batch element per NeuronCore (8 cores).

Per-core computation (batch b), all layouts chosen so every matmul contracts
over the partition dimension with NO on-device transposes (host pre-transposes
the three activation inputs). Matmuls run in bf16 with fp32 PSUM accumulation.

  QT = (Wq^T x^T)           : lhsT=Wq (dm,dout) tiles, rhs=x^T (dm,s)  -> (dout, s)
  KT = (Wk^T x^T)           : same                                      -> (dout, s)
  V  = (x Wv)               : lhsT=x^T (dm,s) tiles,  rhs=Wv (dm,dv)   -> (s, dv)
  logitsT[kpos,q] per head  : lhsT=KhT (d,kpos), rhs=QhT (d,q)  (K=64, two heads
                              packed in PE row-groups 0-63 / 64-127)
  pT = exp(logitsT * 1/8)   : ScalarE; the additive mask is applied
                              multiplicatively to V rows (exp(l+m) = exp(l)*exp(m))
  ctxT'[d+1,q] per head     : lhsT=[Vh | mask'] (kpos,65), rhs=pT (kpos,q)
                              row 64 accumulates the softmax row-sums
  ctxT = ctxT'[0:64] * (1/rowsum broadcast)   (GpSimd partition_broadcast + DVE)
  outA = x Wo[:D] + bo      : accumulated DURING attention (PE is ACT-bound there)
  out  = outA + ctxT^T Wo[D:]

Phase structure (PSUM banks):
  phase 1: projections, 8 held psum groups, k-tiles streamed outer  (8 banks)
  phase 2: attention (4 logits + 2 PV banks) + outA groups (2 banks)
  phase 3: ctx half of the output projection (8 banks)
"""

import os
from contextlib import ExitStack

import numpy as np

import concourse.mybir as mybir
import concourse.tile as tile
from concourse import bacc
from concourse.bass_utils import run_bass_kernel_spmd

F32 = mybir.dt.float32
AF = mybir.ActivationFunctionType

D = 1024          # model dim
S = 1024          # sequence length
H = 16            # heads
DEP = 64          # head dim
B = 8             # batch == n cores
NT = 8            # 128-partition tiles per 1024 dim
SCALE = 1.0 / 8.0  # 1/sqrt(DEP)

# matmul input dtype: bf16 runs the PE at 1 cycle/row with fp32 PSUM
# accumulation and halves DMA traffic. (float32 would be exact but runs the
# PE at 4 cycles/row and its doubled tiles exceed the SBUF residency budget
# of this layout; float32r is full-rate but its fused self-loading matmul
# only admits one semaphore wait, which Tile cannot guarantee.)
_MM_DT_NAME = os.environ.get("MHA_MM_DT", "bf16")
MM_DT = {"bf16": mybir.dt.bfloat16, "f32": mybir.dt.float32}[_MM_DT_NAME]
MM_NP = mybir.dt.np(MM_DT)

LAST_EXEC_NS = None
LAST_RES = None


def _mm(nc, out, lhsT, rhs, start, stop):
    nc.tensor.matmul(out, lhsT, rhs, start=start, stop=stop)


def build_nc(phases=(1, 2, 3)):
    nc = bacc.Bacc(None, target_bir_lowering=False)

    xqT_d = nc.dram_tensor("xqT", [D, S], MM_DT, kind="ExternalInput")
    xkT_d = nc.dram_tensor("xkT", [D, S], MM_DT, kind="ExternalInput")
    xvT_d = nc.dram_tensor("xvT", [D, S], MM_DT, kind="ExternalInput")
    wq_d = nc.dram_tensor("wq", [D, D], MM_DT, kind="ExternalInput")
    wk_d = nc.dram_tensor("wk", [D, D], MM_DT, kind="ExternalInput")
    wv_d = nc.dram_tensor("wv", [D, D], MM_DT, kind="ExternalInput")
    wo_d = nc.dram_tensor("wo", [2 * D, D], MM_DT, kind="ExternalInput")
    bq_d = nc.dram_tensor("bq", [128, NT], F32, kind="ExternalInput")
    bk_d = nc.dram_tensor("bk", [128, NT], F32, kind="ExternalInput")
    bv_d = nc.dram_tensor("bv", [128, D], MM_DT, kind="ExternalInput")
    bo_d = nc.dram_tensor("bo", [128, D], F32, kind="ExternalInput")
    # multiplicative mask exp(-1e9 * mask), laid out [kpos%128, kpos//128]
    mp_d = nc.dram_tensor("mprime", [128, NT], F32, kind="ExternalInput")
    out_d = nc.dram_tensor("out", [S, D], F32, kind="ExternalOutput")

    with tile.TileContext(nc) as tc:
        with (
            tc.tile_pool(name="const", bufs=1) as cp,
            tc.tile_pool(name="qtp", bufs=1) as qtp,
            tc.tile_pool(name="ktp", bufs=1) as ktp,
            tc.tile_pool(name="vp", bufs=1) as vp,
        ):
            # (const DMAs are emitted after the first weight tiles below --
            # the Sync sequencer issues descriptors serially at ~0.65us each
            # and the first matmul only needs w_q_0/x_q_0)
            bq_sb = cp.tile([128, NT], F32, name="bq_sb")
            bk_sb = cp.tile([128, NT], F32, name="bk_sb")
            bv_sb = cp.tile([128, D], MM_DT, name="bv_sb")
            bo_sb = cp.tile([128, D], F32, name="bo_sb")
            mp_sb = cp.tile([128, NT], F32, name="mp_sb")

            # Residents: QT, KT (dout-major) and V65 (s-major, 65 cols/head:
            # 64 of V plus one mask' column that accumulates the softmax sum).
            qt_sb = [qtp.tile([128, S], MM_DT, name=f"qt{i}", tag=f"qt{i}") for i in range(NT)]
            kt_sb = [ktp.tile([128, S], MM_DT, name=f"ktile{i}", tag=f"ktile{i}") for i in range(NT)]
            v65_sb = [vp.tile([128, H, 65], MM_DT, name=f"v65_{i}", tag=f"v65_{i}") for i in range(NT)]

            # ================= Phase 1: projections =================
            # k-tiles outer, held psum groups -> the first matmul only waits
            # for the first w/x tile pair instead of the whole weight matrix.
            # Q/K use all 8 banks; V uses 4 so the logits pool can open early
            # and the first two attention iterations' logits+exp pre-execute
            # during the V projection (head start for the ScalarE exp stream).
            psum_es = ExitStack()
            pre_pts = {}
            lpsp = None

            def emit_logits_exp(dt, qc):
                pair = ((2 * dt, 0), (2 * dt + 1, 64))
                pts = {}
                for h, base in pair:
                    pts[h] = ptp.tile([128, NT, 512], MM_DT, name=f"pt_{dt}_{h}_{qc}", tag=f"pt{h % 2}")
                for g in range(4):
                    lps = {}
                    for h, base in pair:
                        lps[h] = lpsp.tile([128, 1024], F32, name=f"lps_{dt}_{h}_{qc}_{g}", tag=f"lps{h % 2}")
                    for j in range(2):
                        kt = 2 * g + j
                        for h, base in pair:
                            _mm(nc, lps[h][:, j * 512:(j + 1) * 512],
                                kt_sb[dt][base:base + 64, kt * 128:(kt + 1) * 128],
                                qt_sb[dt][base:base + 64, qc * 512:(qc + 1) * 512],
                                start=True, stop=True)
                    for h, base in pair:
                        nc.scalar.activation(
                            pts[h][:, 2 * g:2 * g + 2, :],
                            lps[h].rearrange("p (a c) -> p a c", a=2),
                            AF.Exp, scale=SCALE)
                return pts

            with (
                tc.tile_pool(name="ptp", bufs=3) as ptp,
                tc.tile_pool(name="rp", bufs=2) as rp,
            ):
              with (
                  tc.tile_pool(name="wpool", bufs=1) as wpool,
                  tc.tile_pool(name="xpool", bufs=1) as xpool,
              ):
                def load_wx(w_d, x_d, kind):
                    w_sb = []
                    x_sb = []
                    for kt in range(NT):
                        wt = wpool.tile([128, D], MM_DT, name=f"w_{kind}_{kt}", tag=f"w{kt}")
                        nc.sync.dma_start(wt, w_d[kt * 128:(kt + 1) * 128, :])
                        w_sb.append(wt)
                        xt = xpool.tile([128, S], MM_DT, name=f"x_{kind}_{kt}", tag=f"x{kt}")
                        nc.sync.dma_start(xt, x_d[kt * 128:(kt + 1) * 128, :])
                        x_sb.append(xt)
                        if kind == "q" and kt == 0:
                            nc.sync.dma_start(bq_sb, bq_d[:, :])
                            nc.sync.dma_start(bk_sb, bk_d[:, :])
                            nc.sync.dma_start(bv_sb, bv_d[:, :])
                            nc.sync.dma_start(bo_sb, bo_d[:, :])
                            nc.sync.dma_start(mp_sb, mp_d[:, :])
                    return w_sb, x_sb

                with tc.tile_pool(name="ppsqk", bufs=1, space="PSUM") as ppsqk:
                    for w_d, x_d, kind in ((wq_d, xqT_d, "q"), (wk_d, xkT_d, "k")):
                        if 1 not in phases:
                            break
                        w_sb, x_sb = load_wx(w_d, x_d, kind)
                        dst = qt_sb if kind == "q" else kt_sb
                        bias = bq_sb if kind == "q" else bk_sb
                        for half in range(2):
                            groups = [(a, c) for a in range(half * 4, half * 4 + 4) for c in range(2)]
                            psums = {}
                            for i, g in enumerate(groups):
                                psums[g] = ppsqk.tile([128, 512], F32, name=f"ps_{kind}_{g[0]}_{g[1]}", tag=f"pp{i}")
                            for kt in range(NT):
                                for (a, c) in groups:
                                    _mm(nc, psums[(a, c)],
                                        w_sb[kt][:, a * 128:(a + 1) * 128],
                                        x_sb[kt][:, c * 512:(c + 1) * 512],
                                        start=(kt == 0), stop=(kt == NT - 1))
                            for (dt, sc) in groups:
                                nc.vector.tensor_scalar_add(
                                    dst[dt][:, sc * 512:(sc + 1) * 512], psums[(dt, sc)],
                                    bias[:, dt:dt + 1])

                # logits pool lives from here through the attention loop
                lpsp = psum_es.enter_context(tc.tile_pool(name="lpsp", bufs=1, space="PSUM"))

                with tc.tile_pool(name="ppsv", bufs=1, space="PSUM") as ppsv:
                    if 1 in phases:
                        w_sb, x_sb = load_wx(wv_d, xvT_d, "v")
                        for quarter in range(4):
                            if quarter == 1 and 2 in phases:
                                # PE: V quarter 0 runs while the K-proj psum zone
                                # drains; the pre-attention logits+exp follow with
                                # no stall, giving ScalarE its head start.
                                pre_pts[(0, 0)] = emit_logits_exp(0, 0)
                                pre_pts[(0, 1)] = emit_logits_exp(0, 1)
                            sts = (2 * quarter, 2 * quarter + 1)
                            groups = [(st, c) for st in sts for c in range(2)]
                            psums = {}
                            for i, g in enumerate(groups):
                                psums[g] = ppsv.tile([128, 512], F32, name=f"ps_v_{g[0]}_{g[1]}", tag=f"pv{i}")
                            for kt in range(NT):
                                for (st, c) in groups:
                                    _mm(nc, psums[(st, c)],
                                        x_sb[kt][:, st * 128:(st + 1) * 128],
                                        w_sb[kt][:, c * 512:(c + 1) * 512],
                                        start=(kt == 0), stop=(kt == NT - 1))
                            for st in sts:
                                nc.vector.memset(v65_sb[st][:, :, 64:65], 1.0)
                            for (st, c) in groups:
                                nc.vector.tensor_add(
                                    v65_sb[st][:, c * 8:(c + 1) * 8, 0:64],
                                    psums[(st, c)].rearrange("p (h e) -> p h e", e=64),
                                    bv_sb[:, c * 512:(c + 1) * 512].rearrange("p (h e) -> p h e", e=64))
                            # mask': scale V rows (and the ones col) by exp(-1e9*mask)
                            for st in sts:
                                nc.vector.tensor_scalar_mul(
                                    v65_sb[st][:, :, :], v65_sb[st][:, :, :], mp_sb[:, st:st + 1])

              # ============ Phases 2+3: attention + output projection ============
              with (
                  tc.tile_pool(name="ctxp", bufs=1) as ctxp,
                  tc.tile_pool(name="xq2p", bufs=1) as xq2p,
                  tc.tile_pool(name="wopA", bufs=1) as wopA,
                  tc.tile_pool(name="oap", bufs=1) as oap,
              ):
                  ctx_sb = [ctxp.tile([128, S], MM_DT, name=f"ctx{i}", tag=f"ctx{i}") for i in range(NT)]
                  oa_sb = [oap.tile([128, D], F32, name=f"oa{i}", tag=f"oa{i}") for i in range(NT)]

                  # x^T reload + Wo[:D] tiles: both consumed by outA groups that
                  # run interleaved with attention (PE is ACT-bound there).
                  xq2_sb = []
                  woA_sb = {}
                  if 2 in phases:
                      for kt in range(NT):
                          xt2 = xq2p.tile([128, S], MM_DT, name=f"xq2_{kt}", tag=f"xq2_{kt}")
                          nc.sync.dma_start(xt2, xqT_d[kt * 128:(kt + 1) * 128, :])
                          xq2_sb.append(xt2)
                      for kt2 in range(NT):
                          for dc in range(2):
                              wt = wopA.tile([128, 512], MM_DT, name=f"woA_{kt2}_{dc}", tag=f"woA_{kt2}_{dc}")
                              nc.sync.dma_start(wt, wo_d[kt2 * 128:(kt2 + 1) * 128, dc * 512:(dc + 1) * 512])
                              woA_sb[(kt2, dc)] = wt


                  with (
                      tc.tile_pool(name="pvp", bufs=3, space="PSUM") as pvp,
                      tc.tile_pool(name="oaps", bufs=1, space="PSUM") as oaps,
                  ):
                      for dt in range(NT) if 2 in phases else []:
                          pair = ((2 * dt, 0), (2 * dt + 1, 64))
                          for qc in range(2):
                              pts = pre_pts.pop((dt, qc), None)
                              if pts is None:
                                  pts = emit_logits_exp(dt, qc)
                              for h, base in pair:
                                  pv = pvp.tile([65, 512], F32, name=f"pv_{h}_{qc}", tag="pv")
                                  for kt in range(NT):
                                      _mm(nc, pv, v65_sb[kt][:, h, :], pts[h][:, kt, :],
                                          start=(kt == 0), stop=(kt == NT - 1))
                                  rsum = rp.tile([1, 512], F32, name=f"rsum_{h}_{qc}", tag="rsum")
                                  nc.vector.tensor_copy(rsum, pv[64:65, :])
                                  recip = rp.tile([1, 512], F32, name=f"recip_{h}_{qc}", tag="recip")
                                  # rowsums are positive and well-conditioned; 18-bit
                                  # approx is far below bf16 noise and ~5x faster than
                                  # exact reciprocal (3.3us -> 0.7us on one DVE lane)
                                  nc.vector.reciprocal_approx_fast(recip, rsum)
                                  rbc = rp.tile([64, 512], F32, name=f"rbc_{h}_{qc}", tag="rbc")
                                  nc.gpsimd.partition_broadcast(rbc, recip, channels=64)
                                  nc.vector.tensor_mul(
                                      ctx_sb[dt][base:base + 64, qc * 512:(qc + 1) * 512],
                                      pv[0:64, :], rbc)

                              # one outA group (st, dc) per (dt, qc) iteration:
                              # outA[st, dc] = x Wo_top + bo, using PE slack in the
                              # ACT-bound attention window
                              idx = dt * 2 + qc
                              st, dc = idx // 2, idx % 2
                              psA = oaps.tile([128, 512], F32, name=f"psA_{st}_{dc}", tag="oaps")
                              for kt2 in range(NT):
                                  _mm(nc, psA,
                                      xq2_sb[kt2][:, st * 128:(st + 1) * 128],
                                      woA_sb[(kt2, dc)],
                                      start=(kt2 == 0), stop=(kt2 == NT - 1))
                              nc.vector.tensor_add(
                                  oa_sb[st][:, dc * 512:(dc + 1) * 512], psA,
                                  bo_sb[:, dc * 512:(dc + 1) * 512])

                  # release the logits pool before phase 3 takes all 8 banks
                  psum_es.close()

                  # ========= Phase 3: ctx half of the output projection =========
                  with (
                      tc.tile_pool(name="wopB", bufs=6) as wopB,
                      tc.tile_pool(name="outp", bufs=3) as outp,
                      tc.tile_pool(name="ops", bufs=1, space="PSUM") as ops,
                  ):
                      for dc in range(2) if 3 in phases else []:
                          pso = [ops.tile([128, 512], F32, name=f"pso_{dc}_{st}", tag=f"pso{st}")
                                 for st in range(NT)]
                          for kt2 in range(NT, 2 * NT):
                              wo_t = wopB.tile([128, 512], MM_DT, name=f"woB_{dc}_{kt2}", tag="wo")
                              nc.sync.dma_start(wo_t, wo_d[kt2 * 128:(kt2 + 1) * 128, dc * 512:(dc + 1) * 512])
                              for st in range(NT):
                                  _mm(nc, pso[st],
                                      ctx_sb[kt2 - NT][:, st * 128:(st + 1) * 128],
                                      wo_t,
                                      start=(kt2 == NT), stop=(kt2 == 2 * NT - 1))
                          for st in range(NT):
                              ot = outp.tile([128, 512], F32, name=f"ot_{dc}_{st}", tag="ot")
                              nc.vector.tensor_add(ot, pso[st], oa_sb[st][:, dc * 512:(dc + 1) * 512])
                              nc.sync.dma_start(
                                  out_d[st * 128:(st + 1) * 128, dc * 512:(dc + 1) * 512], ot)

    nc.finalize()
    return nc


_NC_CACHE = None


def _get_nc():
    global _NC_CACHE
    if _NC_CACHE is None:
        _NC_CACHE = build_nc()
    return _NC_CACHE


def kernel(**inputs):
    global LAST_EXEC_NS
    v = np.asarray(inputs["v"], np.float32)
    k = np.asarray(inputs["k"], np.float32)
    q_in = np.asarray(inputs["q_in"], np.float32)
    mask = np.asarray(inputs["mask"], np.float32)
    wq_w = np.asarray(inputs["wq_w"], np.float32)
    wq_b = np.asarray(inputs["wq_b"], np.float32)
    wk_w = np.asarray(inputs["wk_w"], np.float32)
    wk_b = np.asarray(inputs["wk_b"], np.float32)
    wv_w = np.asarray(inputs["wv_w"], np.float32)
    wv_b = np.asarray(inputs["wv_b"], np.float32)
    wo_w = np.asarray(inputs["wo_w"], np.float32)
    wo_b = np.asarray(inputs["wo_b"], np.float32)

    bq = np.ascontiguousarray(wq_b.reshape(NT, 128).T)
    bk = np.ascontiguousarray(wk_b.reshape(NT, 128).T)
    bv = np.ascontiguousarray(np.broadcast_to(wv_b, (128, D))).astype(MM_NP)
    bo = np.ascontiguousarray(np.broadcast_to(wo_b, (128, D)))
    wq_m = wq_w.astype(MM_NP)
    wk_m = wk_w.astype(MM_NP)
    wv_m = wv_w.astype(MM_NP)
    wo_m = wo_w.astype(MM_NP)

    in_maps = []
    for b in range(B):
        mcol = np.exp(np.float32(-1e9) * mask[b, 0, 0, :]).astype(np.float32)
        in_maps.append({
            "xqT": np.ascontiguousarray(q_in[b].T.astype(MM_NP)),
            "xkT": np.ascontiguousarray(k[b].T.astype(MM_NP)),
            "xvT": np.ascontiguousarray(v[b].T.astype(MM_NP)),
            "wq": wq_m, "wk": wk_m, "wv": wv_m, "wo": wo_m,
            "bq": bq, "bk": bk, "bv": bv, "bo": bo,
            "mprime": np.ascontiguousarray(mcol.reshape(NT, 128).T),
        })

    nc = _get_nc()
    trace = os.environ.get("MHA_TRACE", "0") == "1"
    res = run_bass_kernel_spmd(nc, in_maps, core_ids=list(range(B)), trace=trace)
    LAST_EXEC_NS = res.exec_time_ns
    globals()["LAST_RES"] = res
    return np.stack([r["out"] for r in res.results], axis=0)



# revision 17
# speedup vs baseline: 1.7152x; 1.7152x over previous
"""Trainium2 Bass kernel for nn_MultiHeadAttention_73607149519012.

MHA: B=8, S=1024, D=1024, H=16 heads, depth=64, fp32 in/out.
Sharding: data-parallel over batch -- one batch element per NeuronCore (8 cores).

v2: fp8e4 DoubleRow matmuls everywhere except the dominant q_in @ Wo_top half
(bf16).  DoubleRow packs two K-tiles per instruction at 0.5 cycles/row, so the
fp8 matmuls run ~4x faster than bf16 per MAC.  The attention path contributes
only ~2% of the output magnitude (ctx std ~0.02 vs q_in std ~1), so fp8 noise
there is diluted ~50x and the dominant half stays bf16.

Layouts (all host-prepared, no on-device transposes):
  Q/K proj : psum[dout',s] = Wq'^T x^T  with Wq' column-permuted so each psum
             chunk drains 1:1 into the DoubleRow logits layout
             qt[tile][32*(h%4)+d%32, d//32, s]  (4 heads per 128-partition tile)
  V proj   : psum[s,dv] -> v65p[kt//2][s, kt%2, head, 0:64]; column 64 holds
             mask[kpos]/32 so the PV matmul accumulates rowsum/32 in row 64
  logits   : DR lhsT=KT[32,2,128], rhs=QT[32,2,512] at tile_position (32*(h%4),0)
             -> psum[kpos128, q512];  exp has NO mask term (mask rides V rows)
  exp      : split across engines: exact Exp on ACT; Schraudolph bit-trick
             int8(a*x + b) bitcast as fp8e4 on DVE/Pool (rms ~3%, mean ~0)
  PV       : DR lhsT=v65p[128,2,65], rhs=pts[128,2,512] -> psum[65, q512];
             row 64 = rowsum/32; recip -> broadcast -> mul gives ctx*32 in fp8
  out      : psum[s,dout] = x@Wo_top (bf16, + K=1 ones-row matmul adding bo)
             drained to bf16 oa; then psum2 = (32ctx)@(64Wo_bot) (fp8 DR) and
             final out = psum2/2048 + oa  (scalar_tensor_tensor)
"""

import os
from contextlib import ExitStack

import numpy as np
import ml_dtypes

import concourse.mybir as mybir
import concourse.tile as tile
from concourse import bacc
from concourse.bass_utils import run_bass_kernel_spmd

F32 = mybir.dt.float32
BF16 = mybir.dt.bfloat16
F8 = mybir.dt.float8e4
I8 = mybir.dt.int8
AF = mybir.ActivationFunctionType
ALU = mybir.AluOpType
DR = mybir.MatmulPerfMode.DoubleRow

NP8 = ml_dtypes.float8_e4m3
NPBF = ml_dtypes.bfloat16

D = 1024
S = 1024
H = 16
DEP = 64
B = 8
SCALE = 1.0 / 8.0          # 1/sqrt(DEP)
A_SCH = 8.0 / np.log(2.0)  # Schraudolph slope for 3-bit-mantissa fp8
B_SCH = 56.0 - 0.45        # bias 7*8, calibrated -0.45 to zero the mean error
CTX_S = 32.0               # ctx scaled by 32 into fp8 (ones column = 1/32)
WOB_S = 64.0               # Wo_bot scaled by 64 into fp8
OUT_S = 1.0 / (CTX_S * WOB_S)

# exp engine split (ACT exact, DVE Schraudolph; Pool cannot read PSUM)
_EW = os.environ.get("MHA_EXP_W", "83,45,0")
EXP_W = tuple(int(x) for x in _EW.split(","))

LAST_EXEC_NS = None
LAST_RES = None


def _exp_schedule():
    """128 exp units -> engine labels by largest-remainder apportionment,
    interleaved so consecutive units rotate engines."""
    total = sum(EXP_W)
    counts = [w * 128 // total for w in EXP_W]
    while sum(counts) < 128:
        counts[int(np.argmax([w / (c + 1) for w, c in zip(EXP_W, counts)]))] += 1
    sched = []
    acc = [0.0, 0.0, 0.0]
    for _ in range(128):
        for i in range(3):
            acc[i] += counts[i] / 128.0
        pick = int(np.argmax(acc))
        acc[pick] -= 1.0
        sched.append(pick)
    return sched


def build_nc():
    nc = bacc.Bacc(None, target_bir_lowering=False)

    xq8_d = nc.dram_tensor("xq8", [128, 4, 2, S], F8, kind="ExternalInput")
    xk8_d = nc.dram_tensor("xk8", [128, 4, 2, S], F8, kind="ExternalInput")
    xv8_d = nc.dram_tensor("xv8", [128, 4, 2, S], F8, kind="ExternalInput")
    wq8_d = nc.dram_tensor("wq8", [128, 4, 2, D], F8, kind="ExternalInput")
    wk8_d = nc.dram_tensor("wk8", [128, 4, 2, D], F8, kind="ExternalInput")
    wv8_d = nc.dram_tensor("wv8", [128, 4, 2, D], F8, kind="ExternalInput")
    wob8_d = nc.dram_tensor("wob8", [128, 4, 2, D], F8, kind="ExternalInput")
    xqb_d = nc.dram_tensor("xqb", [128, 8, S], BF16, kind="ExternalInput")
    wot_d = nc.dram_tensor("wot", [128, 8, D], BF16, kind="ExternalInput")
    # cst cols: 0-7 bq' (permuted), 8-15 bk' (permuted), 16-23 mprime per st
    cst_d = nc.dram_tensor("cst", [128, 24], F32, kind="ExternalInput")
    bvm_d = nc.dram_tensor("bvm", [128, 8, D], BF16, kind="ExternalInput")
    bor_d = nc.dram_tensor("bor", [1, D], BF16, kind="ExternalInput")
    out_d = nc.dram_tensor("out", [S, D], F32, kind="ExternalOutput")

    dbg = os.environ.get("MHA_DBG", "0") == "1"
    if dbg:
        dqt_d = nc.dram_tensor("dqt", [4, 128, 2, S], F8, kind="ExternalOutput")
        dkt_d = nc.dram_tensor("dkt", [4, 128, 2, S], F8, kind="ExternalOutput")
        dv65_d = nc.dram_tensor("dv65", [4, 128, 2, H, 65], F8, kind="ExternalOutput")
        dpts_d = nc.dram_tensor("dpts", [128, 4, 2, S], F8, kind="ExternalOutput")
        dpv_d = nc.dram_tensor("dpv", [2, 65, 512], F32, kind="ExternalOutput")
        drc_d = nc.dram_tensor("drc", [2, 1, 512], F32, kind="ExternalOutput")
        drb_d = nc.dram_tensor("drb", [2, 64, 512], F32, kind="ExternalOutput")
        dctx_d = nc.dram_tensor("dctx", [4, 128, 2, S], F8, kind="ExternalOutput")
        doa_d = nc.dram_tensor("doa", [8, 128, D], BF16, kind="ExternalOutput")

    exp_sched = _exp_schedule()
    exp_i = [0]

    with tile.TileContext(nc) as tc:
        es = ExitStack()
        cp = es.enter_context(tc.tile_pool(name="cp", bufs=1))
        ap_es = ExitStack()
        ap = ap_es.enter_context(tc.tile_pool(name="ap", bufs=1))  # phase-A inputs

        # ---- persistent tiles ----
        cst = cp.tile([128, 24], F32, name="cst")
        xqb = cp.tile([128, 8, S], BF16, name="xqb")
        wot = cp.tile([128, 8, D], BF16, name="wot")
        bvm = cp.tile([128, 8, D], BF16, name="bvm")
        bor = cp.tile([1, D], BF16, name="bor")
        ones1 = cp.tile([1, 128], BF16, name="ones1")
        qt = [cp.tile([128, 2, S], F8, name=f"qt{t}", tag=f"qt{t}") for t in range(4)]
        kt = [cp.tile([128, 2, S], F8, name=f"kt{t}", tag=f"kt{t}") for t in range(4)]
        v65 = [cp.tile([128, 2, H, 65], F8, name=f"v65_{t}", tag=f"v65_{t}") for t in range(4)]
        ctxp = [cp.tile([128, 2, S], F8, name=f"ctx{t}", tag=f"ctx{t}") for t in range(4)]
        oa = [cp.tile([128, D], BF16, name=f"oa{t}", tag=f"oa{t}") for t in range(8)]
        wob8 = cp.tile([128, 4, 2, D], F8, name="wob8")

        if True:
            xq8 = ap.tile([128, 4, 2, S], F8, name="xq8")
            wq8 = ap.tile([128, 4, 2, D], F8, name="wq8")
            xk8 = ap.tile([128, 4, 2, S], F8, name="xk8")
            wk8 = ap.tile([128, 4, 2, D], F8, name="wk8")
            xv8 = ap.tile([128, 4, 2, S], F8, name="xv8")
            wv8 = ap.tile([128, 4, 2, D], F8, name="wv8")

            # loads in need-order; SP streams ahead of compute
            nc.sync.dma_start(cst, cst_d[:, :])
            nc.sync.dma_start(xq8, xq8_d[:, :, :, :])
            nc.sync.dma_start(wq8, wq8_d[:, :, :, :])
            nc.sync.dma_start(xk8, xk8_d[:, :, :, :])
            nc.sync.dma_start(wk8, wk8_d[:, :, :, :])
            nc.sync.dma_start(xv8, xv8_d[:, :, :, :])
            nc.sync.dma_start(wv8, wv8_d[:, :, :, :])
            nc.sync.dma_start(bvm, bvm_d[:, :, :])
            nc.sync.dma_start(xqb, xqb_d[:, :, :])
            nc.sync.dma_start(wot, wot_d[:, :, :])
            nc.sync.dma_start(bor, bor_d[:, :])
            nc.sync.dma_start(wob8, wob8_d[:, :, :, :])
            nc.vector.memset(ones1, 1.0)

            # ones/mask columns of v65 (column 64 = mprime/32 per kpos)
            for t in range(4):
                nc.vector.memset(v65[t][:, :, :, 64:65], 1.0 / CTX_S)
            for st in range(8):
                t, b = st // 2, st % 2
                nc.vector.tensor_scalar_mul(
                    v65[t][:, b, :, 64:65], v65[t][:, b, :, 64:65],
                    cst[:, 16 + st:17 + st])

            psum_es = ExitStack()
            oap = psum_es.enter_context(tc.tile_pool(name="oap", bufs=2, space="PSUM"))

            def emit_outA(gi):
                st, dc = gi // 2, gi % 2
                psA = oap.tile([128, 512], F32, name=f"psA_{st}_{dc}", tag="oap")
                for t8 in range(8):
                    nc.tensor.matmul(psA, xqb[:, t8, st * 128:(st + 1) * 128],
                                     wot[:, t8, dc * 512:(dc + 1) * 512],
                                     start=(t8 == 0), stop=False)
                nc.tensor.matmul(psA, ones1, bor[:, dc * 512:(dc + 1) * 512],
                                 start=False, stop=True)
                nc.scalar.activation(oa[st][:, dc * 512:(dc + 1) * 512], psA, AF.Copy)

            # ================= Phase A: projections =================
            with tc.tile_pool(name="ppA", bufs=4, space="PSUM") as ppA:
                for kind in ("q", "k"):
                    w_sb = wq8 if kind == "q" else wk8
                    x_sb = xq8 if kind == "q" else xk8
                    dst = qt if kind == "q" else kt
                    bcol = 0 if kind == "q" else 8
                    for a in range(8):
                        tq, bq = a // 2, a % 2
                        for sc in range(2):
                            ps = ppA.tile([128, 512], F32, name=f"ps_{kind}_{a}_{sc}", tag="ppA")
                            for t in range(4):
                                nc.tensor.matmul(
                                    ps, w_sb[:, t, :, a * 128:(a + 1) * 128],
                                    x_sb[:, t, :, sc * 512:(sc + 1) * 512],
                                    start=(t == 0), stop=(t == 3), perf_mode=DR)
                            nc.scalar.activation(
                                dst[tq][:, bq, sc * 512:(sc + 1) * 512], ps,
                                AF.Identity, bias=cst[:, bcol + a:bcol + a + 1])
                for st in range(8):
                    t, b = st // 2, st % 2
                    for c in range(2):
                        ps = ppA.tile([128, 512], F32, name=f"ps_v_{st}_{c}", tag="ppA")
                        for t4 in range(4):
                            nc.tensor.matmul(
                                ps, xv8[:, t4, :, st * 128:(st + 1) * 128],
                                wv8[:, t4, :, c * 512:(c + 1) * 512],
                                start=(t4 == 0), stop=(t4 == 3), perf_mode=DR)
                        nc.vector.scalar_tensor_tensor(
                            v65[t][:, b, c * 8:(c + 1) * 8, 0:64],
                            ps.rearrange("p (h e) -> p h e", e=64),
                            cst[:, 16 + st:17 + st],
                            bvm[:, st, c * 512:(c + 1) * 512].rearrange("p (h e) -> p h e", e=64),
                            op0=ALU.mult, op1=ALU.add)
        ap_es.close()
        if dbg:
            for t in range(4):
                nc.sync.dma_start(dqt_d[t], qt[t])
                nc.sync.dma_start(dkt_d[t], kt[t])
                nc.sync.dma_start(dv65_d[t], v65[t])

        # ============ Phase B: attention ============
        next_g = [0]
        with (
            tc.tile_pool(name="ptsp", bufs=2) as ptsp,
            tc.tile_pool(name="rp", bufs=3) as rp,
            tc.tile_pool(name="rbp", bufs=3) as rbp,
            tc.tile_pool(name="lpsp", bufs=2, space="PSUM") as lpsp,
            tc.tile_pool(name="pvp", bufs=2, space="PSUM") as pvp,
        ):
            for h in range(H):
                tq, jj = h // 4, h % 4
                base = 32 * jj
                pts = ptsp.tile([128, 4, 2, S], F8, name=f"pts{h}", tag="pts")
                for qc in range(2):
                    for j in range(4):
                        lps = lpsp.tile([128, 2, 512], F32, name=f"lps_{h}_{qc}_{j}", tag="lps")
                        for kk in range(2):
                            kc = 2 * j + kk
                            nc.tensor.matmul(
                                lps[:, kk, :],
                                kt[tq][base:base + 32, :, kc * 128:(kc + 1) * 128],
                                qt[tq][base:base + 32, :, qc * 512:(qc + 1) * 512],
                                start=True, stop=True, perf_mode=DR,
                                tile_position=(base, 0))
                        eng = exp_sched[exp_i[0] % 128]
                        exp_i[0] += 1
                        dst = pts[:, j, :, qc * 512:(qc + 1) * 512]
                        if eng == 0:
                            nc.scalar.activation(dst, lps, AF.Exp, scale=SCALE)
                        else:
                            e = nc.vector if eng == 1 else nc.gpsimd
                            e.tensor_scalar(dst.bitcast(I8), lps,
                                            A_SCH * SCALE, B_SCH,
                                            op0=ALU.mult, op1=ALU.add)
                    pv = pvp.tile([65, 512], F32, name=f"pv_{h}_{qc}", tag="pv")
                    for t in range(4):
                        nc.tensor.matmul(pv, v65[t][:, :, h, :],
                                         pts[:, t, :, qc * 512:(qc + 1) * 512],
                                         start=(t == 0), stop=(t == 3), perf_mode=DR)
                    recip = rp.tile([1, 512], F32, name=f"rc_{h}_{qc}", tag="rc")
                    nc.vector.reciprocal(recip, pv[64:65, :])
                    rbc = rbp.tile([64, 512], F32, name=f"rb_{h}_{qc}", tag="rb")
                    nc.gpsimd.partition_broadcast(rbc, recip, channels=64)
                    ct, cb = h // 4, (h // 2) % 2
                    nc.vector.tensor_mul(
                        ctxp[ct][64 * (h % 2):64 * (h % 2) + 64, cb, qc * 512:(qc + 1) * 512],
                        pv[0:64, :], rbc)
                    if dbg and h == 0:
                        pvc = rp.tile([65, 512], F32, name=f"dbgpv{qc}", tag="dbgpv")
                        nc.vector.tensor_copy(pvc, pv)
                        nc.sync.dma_start(dpv_d[qc], pvc)
                        nc.sync.dma_start(drc_d[qc], recip)
                        nc.sync.dma_start(drb_d[qc], rbc)
                if dbg and h == 0:
                    nc.sync.dma_start(dpts_d[:, :, :, :], pts)
                while next_g[0] < h + 1:
                    emit_outA(next_g[0])
                    next_g[0] += 1

        psum_es.close()
        if dbg:
            for t in range(4):
                nc.sync.dma_start(dctx_d[t], ctxp[t])
            for st in range(8):
                nc.sync.dma_start(doa_d[st], oa[st])

        # ============ Phase C: ctx @ Wo_bot + combine ============
        with (
            tc.tile_pool(name="outp", bufs=2) as outp,
            tc.tile_pool(name="php", bufs=3, space="PSUM") as php,
        ):
            for st in range(8):
                osb = outp.tile([128, D], F32, name=f"osb{st}", tag="osb")
                for dc in range(2):
                    ps2 = php.tile([128, 512], F32, name=f"ps2_{st}_{dc}", tag="php")
                    for j in range(4):
                        nc.tensor.matmul(ps2, ctxp[j][:, :, st * 128:(st + 1) * 128],
                                         wob8[:, j, :, dc * 512:(dc + 1) * 512],
                                         start=(j == 0), stop=(j == 3), perf_mode=DR)
                    nc.vector.scalar_tensor_tensor(
                        osb[:, dc * 512:(dc + 1) * 512], ps2, OUT_S,
                        oa[st][:, dc * 512:(dc + 1) * 512],
                        op0=ALU.mult, op1=ALU.add)
                nc.sync.dma_start(out_d[st * 128:(st + 1) * 128, :], osb)

        es.close()

    nc.finalize()
    return nc


_NC_CACHE = None


def _get_nc():
    global _NC_CACHE
    if _NC_CACHE is None:
        _NC_CACHE = build_nc()
    return _NC_CACHE


def _perm():
    # pi[a*128 + q] = dout packed at (tile a//2, block a%2, partition 32*(q//32)... )
    pi = np.empty(D, np.int64)
    for a in range(8):
        for q in range(128):
            pi[a * 128 + q] = 64 * (4 * (a // 2) + q // 32) + 32 * (a % 2) + q % 32
    return pi


def _pair4(x):
    # [1024, N] -> [128, 4, 2, N] with row 128*(2t+b)+p at [p, t, b]
    n = x.shape[1]
    return np.ascontiguousarray(x.reshape(4, 2, 128, n).transpose(2, 0, 1, 3))


def kernel(**inputs):
    global LAST_EXEC_NS
    v = np.asarray(inputs["v"], np.float32)
    k = np.asarray(inputs["k"], np.float32)
    q_in = np.asarray(inputs["q_in"], np.float32)
    mask = np.asarray(inputs["mask"], np.float32)
    wq_w = np.asarray(inputs["wq_w"], np.float32)
    wq_b = np.asarray(inputs["wq_b"], np.float32)
    wk_w = np.asarray(inputs["wk_w"], np.float32)
    wk_b = np.asarray(inputs["wk_b"], np.float32)
    wv_w = np.asarray(inputs["wv_w"], np.float32)
    wv_b = np.asarray(inputs["wv_b"], np.float32)
    wo_w = np.asarray(inputs["wo_w"], np.float32)
    wo_b = np.asarray(inputs["wo_b"], np.float32)

    pi = _perm()
    wq8 = _pair4(wq_w[:, pi].astype(NP8))
    wk8 = _pair4(wk_w[:, pi].astype(NP8))
    wv8 = _pair4(wv_w.astype(NP8))
    wob8 = _pair4((wo_w[D:] * WOB_S).astype(NP8))
    wot = np.ascontiguousarray(
        wo_w[:D].reshape(8, 128, D).transpose(1, 0, 2)).astype(NPBF)
    bor = wo_b.reshape(1, D).astype(NPBF)
    bqp = wq_b[pi].reshape(8, 128).T          # [128, 8]
    bkp = wk_b[pi].reshape(8, 128).T

    in_maps = []
    for bi in range(B):
        m = np.exp(np.float32(-1e9) * mask[bi, 0, 0, :]).astype(np.float32)
        m_st = m.reshape(8, 128).T            # [128, 8]
        cst = np.concatenate([bqp, bkp, m_st], axis=1).astype(np.float32)
        bvm = (wv_b[None, None, :] * m_st[:, :, None]).astype(NPBF)  # [128, 8, 1024]
        xqT = np.ascontiguousarray(q_in[bi].T)
        in_maps.append({
            "xq8": _pair4(xqT.astype(NP8)),
            "xk8": _pair4(k[bi].T.astype(NP8)),
            "xv8": _pair4(v[bi].T.astype(NP8)),
            "wq8": wq8, "wk8": wk8, "wv8": wv8, "wob8": wob8,
            "xqb": np.ascontiguousarray(xqT.reshape(8, 128, S).transpose(1, 0, 2)).astype(NPBF),
            "wot": wot, "cst": np.ascontiguousarray(cst),
            "bvm": np.ascontiguousarray(bvm), "bor": bor,
        })

    nc = _get_nc()
    trace = os.environ.get("MHA_TRACE", "0") == "1"
    res = run_bass_kernel_spmd(nc, in_maps, core_ids=list(range(B)), trace=trace)
    LAST_EXEC_NS = res.exec_time_ns
    globals()["LAST_RES"] = res
    return np.stack([r["out"] for r in res.results], axis=0)


# revision 20
# speedup vs baseline: 1.7751x; 1.0349x over previous
"""Trainium2 Bass kernel for nn_MultiHeadAttention_73607149519012.

MHA: B=8, S=1024, D=1024, H=16 heads, depth=64, fp32 in/out.
Sharding: data-parallel over batch -- one batch element per NeuronCore (8 cores).

fp8e4 DoubleRow matmuls everywhere except the dominant q_in @ Wo_top half
(bf16).  DoubleRow packs two K-tiles per instruction at 0.5 cycles/row, so the
fp8 matmuls run ~4x faster than bf16 per MAC.  The attention path contributes
only ~2% of the output magnitude (ctx std ~0.02 vs q_in std ~1), so fp8 noise
there is diluted ~50x and the dominant half stays bf16.

Layouts (all host-prepared, no on-device transposes):
  Q/K proj : psum[dout',s] = Wq'^T x^T  with Wq' column-permuted so each psum
             chunk drains 1:1 into the DoubleRow logits layout
             qt[tile][32*(h%4)+d%32, d//32, s]  (4 heads per 128-partition tile)
  V proj   : psum[s,dv] -> v65p[kt//2][s, kt%2, head, 0:64]; column 64 holds
             mask[kpos]/32 so the PV matmul accumulates rowsum/32 in row 64
  logits   : DR lhsT=KT[32,2,128], rhs=QT[32,2,512] at tile_position (32*(h%4),0)
             -> psum[kpos128, q512];  exp has NO mask term (mask rides V rows)
  exp      : split ACT (exact Exp) / DVE (Schraudolph int8(a*x+b) bitcast fp8,
             rms ~3%, mean ~0; GPSIMD cannot read PSUM so Pool gets none)
  PV       : DR lhsT=v65p[128,2,65], rhs=pts[128,2,512] -> psum[65, 2, q512]
             (both q-chunks in one 2-bank tile); row 64 = rowsum/32;
             reciprocal -> Pool broadcast -> DVE mul gives ctx*32 in fp8
  out      : psum[s,dout] = x@Wo_top (bf16, + K=1 ones-row matmul adding bo)
             drained by ACT to bf16 oa; then psum2 = (32ctx)@(64Wo_bot) fp8 DR
             and final out = psum2/2048 + oa  (scalar_tensor_tensor)

Emission interleaves phases so the exp stream (the ACT/DVE bottleneck) starts
as early as possible: V proj first, then per-tile [Q chunks, K chunks, 4 heads].
"""

import os
from contextlib import ExitStack

import numpy as np
import ml_dtypes

import concourse.mybir as mybir
import concourse.tile as tile
from concourse import bacc
from concourse.bass_utils import run_bass_kernel_spmd

F32 = mybir.dt.float32
BF16 = mybir.dt.bfloat16
F8 = mybir.dt.float8e4
I8 = mybir.dt.int8
AF = mybir.ActivationFunctionType
ALU = mybir.AluOpType
DR = mybir.MatmulPerfMode.DoubleRow

NP8 = ml_dtypes.float8_e4m3
NPBF = ml_dtypes.bfloat16

D = 1024
S = 1024
H = 16
DEP = 64
B = 8
SCALE = 1.0 / 8.0          # 1/sqrt(DEP)
A_SCH = 8.0 / np.log(2.0)  # Schraudolph slope for 3-bit-mantissa fp8
B_SCH = 56.0 - 0.45        # bias 7*8, calibrated -0.45 to zero the mean error
CTX_S = 32.0               # ctx scaled by 32 into fp8 (ones column = 1/32)
WOB_S = 64.0               # Wo_bot scaled by 64 into fp8
OUT_S = 1.0 / (CTX_S * WOB_S)

# exp engine split ACT:DVE:(Pool unused), tunable for balance
_EW = os.environ.get("MHA_EXP_W", "83,45,0")
EXP_W = tuple(int(x) for x in _EW.split(","))

LAST_EXEC_NS = None
LAST_RES = None


def _exp_schedule():
    total = sum(EXP_W)
    counts = [w * 128 // total for w in EXP_W]
    while sum(counts) < 128:
        counts[int(np.argmax([w / (c + 1) for w, c in zip(EXP_W, counts)]))] += 1
    sched = []
    acc = [0.0, 0.0, 0.0]
    for _ in range(128):
        for i in range(3):
            acc[i] += counts[i] / 128.0
        pick = int(np.argmax(acc))
        acc[pick] -= 1.0
        sched.append(pick)
    return sched


def build_nc():
    nc = bacc.Bacc(None, target_bir_lowering=False)

    xq8_d = nc.dram_tensor("xq8", [128, 4, 2, S], F8, kind="ExternalInput")
    xk8_d = nc.dram_tensor("xk8", [128, 4, 2, S], F8, kind="ExternalInput")
    xv8_d = nc.dram_tensor("xv8", [128, 4, 2, S], F8, kind="ExternalInput")
    wq8_d = nc.dram_tensor("wq8", [128, 4, 2, D], F8, kind="ExternalInput")
    wk8_d = nc.dram_tensor("wk8", [128, 4, 2, D], F8, kind="ExternalInput")
    wv8_d = nc.dram_tensor("wv8", [128, 4, 2, D], F8, kind="ExternalInput")
    wob8_d = nc.dram_tensor("wob8", [128, 4, 2, D], F8, kind="ExternalInput")
    xqb_d = nc.dram_tensor("xqb", [128, 8, S], BF16, kind="ExternalInput")
    wot_d = nc.dram_tensor("wot", [128, 8, D], BF16, kind="ExternalInput")
    # cst cols: 0-7 bq' (permuted), 8-15 bk' (permuted), 16-23 mprime per st
    cst_d = nc.dram_tensor("cst", [128, 24], F32, kind="ExternalInput")
    bvm_d = nc.dram_tensor("bvm", [128, 8, D], F8, kind="ExternalInput")
    bor_d = nc.dram_tensor("bor", [1, D], BF16, kind="ExternalInput")
    out_d = nc.dram_tensor("out", [S, D], F32, kind="ExternalOutput")

    dbg = os.environ.get("MHA_DBG", "0") == "1"
    if dbg:
        dqt_d = nc.dram_tensor("dqt", [4, 128, 2, S], F8, kind="ExternalOutput")
        dkt_d = nc.dram_tensor("dkt", [4, 128, 2, S], F8, kind="ExternalOutput")
        dv65_d = nc.dram_tensor("dv65", [4, 128, 2, H, 65], F8, kind="ExternalOutput")
        dpts_d = nc.dram_tensor("dpts", [128, 4, 2, S], F8, kind="ExternalOutput")
        dctx_d = nc.dram_tensor("dctx", [4, 128, 2, S], F8, kind="ExternalOutput")
        doa_d = nc.dram_tensor("doa", [8, 128, D], BF16, kind="ExternalOutput")

    exp_sched = _exp_schedule()
    exp_i = [0]

    with tile.TileContext(nc) as tc:
        es = ExitStack()
        cp = es.enter_context(tc.tile_pool(name="cp", bufs=1))
        ap_es = ExitStack()
        ap = ap_es.enter_context(tc.tile_pool(name="ap", bufs=1))

        # ---- persistent tiles ----
        cst = cp.tile([128, 24], F32, name="cst")
        xqb = cp.tile([128, 8, S], BF16, name="xqb")
        wot = cp.tile([128, 8, D], BF16, name="wot")
        bvm = cp.tile([128, 8, D], F8, name="bvm")
        bor = cp.tile([1, D], BF16, name="bor")
        ones1 = cp.tile([1, 128], BF16, name="ones1")
        qt = [cp.tile([128, 2, S], F8, name=f"qt{t}", tag=f"qt{t}") for t in range(4)]
        kt = [cp.tile([128, 2, S], F8, name=f"kt{t}", tag=f"kt{t}") for t in range(4)]
        v65 = [cp.tile([128, 2, H, 65], F8, name=f"v65_{t}", tag=f"v65_{t}") for t in range(4)]
        ctxp = [cp.tile([128, 2, S], F8, name=f"ctx{t}", tag=f"ctx{t}") for t in range(4)]
        oa = [cp.tile([128, D], BF16, name=f"oa{t}", tag=f"oa{t}") for t in range(8)]

        xv8 = ap.tile([128, 4, 2, S], F8, name="xv8")
        wv8 = ap.tile([128, 4, 2, D], F8, name="wv8")
        xq8 = ap.tile([128, 4, 2, S], F8, name="xq8")
        wq8 = ap.tile([128, 4, 2, D], F8, name="wq8")
        xk8 = ap.tile([128, 4, 2, S], F8, name="xk8")
        wk8 = ap.tile([128, 4, 2, D], F8, name="wk8")

        # loads in need-order; SP streams ahead of compute
        nc.sync.dma_start(cst, cst_d[:, :])
        nc.sync.dma_start(xv8, xv8_d[:, :, :, :])
        nc.sync.dma_start(wv8, wv8_d[:, :, :, :])
        nc.sync.dma_start(bvm, bvm_d[:, :, :])
        nc.sync.dma_start(xq8, xq8_d[:, :, :, :])
        nc.sync.dma_start(wq8, wq8_d[:, :, :, :])
        nc.sync.dma_start(xk8, xk8_d[:, :, :, :])
        nc.sync.dma_start(wk8, wk8_d[:, :, :, :])
        nc.sync.dma_start(xqb, xqb_d[:, :, :])
        nc.sync.dma_start(wot, wot_d[:, :, :])
        nc.sync.dma_start(bor, bor_d[:, :])
        nc.gpsimd.memset(ones1, 1.0)

        # ones/mask columns of v65 (column 64 = mprime/32 per kpos), on Pool
        for t in range(4):
            nc.gpsimd.memset(v65[t][:, :, :, 64:65], 1.0 / CTX_S)
        for st in range(8):
            t, b = st // 2, st % 2
            nc.gpsimd.tensor_scalar_mul(
                v65[t][:, b, :, 64:65], v65[t][:, b, :, 64:65],
                cst[:, 16 + st:17 + st])

        psum_es = ExitStack()
        gp = psum_es.enter_context(tc.tile_pool(name="gp", bufs=2, space="PSUM"))

        def emit_outA(gi):
            st, dc = gi // 2, gi % 2
            psA = gp.tile([128, 512], F32, name=f"psA_{st}_{dc}", tag="gp")
            for t8 in range(8):
                nc.tensor.matmul(psA, xqb[:, t8, st * 128:(st + 1) * 128],
                                 wot[:, t8, dc * 512:(dc + 1) * 512],
                                 start=(t8 == 0), stop=False)
            nc.tensor.matmul(psA, ones1, bor[:, dc * 512:(dc + 1) * 512],
                             start=False, stop=True)
            nc.scalar.activation(oa[st][:, dc * 512:(dc + 1) * 512], psA, AF.Copy)

        # ---- V projection (own psum pool, closed before attention opens) ----
        with tc.tile_pool(name="ppV", bufs=4, space="PSUM") as ppV:
            for st in range(8):
                t, b = st // 2, st % 2
                for c in range(2):
                    ps = ppV.tile([128, 512], F32, name=f"ps_v_{st}_{c}", tag="ppV")
                    for t4 in range(4):
                        nc.tensor.matmul(
                            ps, xv8[:, t4, :, st * 128:(st + 1) * 128],
                            wv8[:, t4, :, c * 512:(c + 1) * 512],
                            start=(t4 == 0), stop=(t4 == 3), perf_mode=DR)
                    nc.vector.scalar_tensor_tensor(
                        v65[t][:, b, c * 8:(c + 1) * 8, 0:64],
                        ps.rearrange("p (h e) -> p h e", e=64),
                        cst[:, 16 + st:17 + st],
                        bvm[:, st, c * 512:(c + 1) * 512].rearrange("p (h e) -> p h e", e=64),
                        op0=ALU.mult, op1=ALU.add)

        # ---- interleaved Q/K projections + attention, by head-group tile ----
        with (
            tc.tile_pool(name="ptsp", bufs=2) as ptsp,
            tc.tile_pool(name="rp", bufs=2) as rp,
            tc.tile_pool(name="rbp", bufs=2) as rbp,
            tc.tile_pool(name="lpsp", bufs=2, space="PSUM") as lpsp,
            tc.tile_pool(name="pvp", bufs=1, space="PSUM") as pvp,
        ):
            for tg in range(4):
                for kind in ("q", "k"):
                    w_sb = wq8 if kind == "q" else wk8
                    x_sb = xq8 if kind == "q" else xk8
                    dst = qt if kind == "q" else kt
                    bcol = 0 if kind == "q" else 8
                    for a in (2 * tg, 2 * tg + 1):
                        for sc in range(2):
                            ps = gp.tile([128, 512], F32, name=f"ps_{kind}_{a}_{sc}", tag="gp")
                            for t in range(4):
                                nc.tensor.matmul(
                                    ps, w_sb[:, t, :, a * 128:(a + 1) * 128],
                                    x_sb[:, t, :, sc * 512:(sc + 1) * 512],
                                    start=(t == 0), stop=(t == 3), perf_mode=DR)
                            nc.scalar.activation(
                                dst[a // 2][:, a % 2, sc * 512:(sc + 1) * 512], ps,
                                AF.Identity, bias=cst[:, bcol + a:bcol + a + 1])
                for h in range(4 * tg, 4 * tg + 4):
                    jj = h % 4
                    base = 32 * jj
                    pts = ptsp.tile([128, 4, 2, S], F8, name=f"pts{h}", tag="pts")
                    for qc in range(2):
                        for j in range(4):
                            lps = lpsp.tile([128, 2, 512], F32, name=f"lps_{h}_{qc}_{j}", tag="lps")
                            for kk in range(2):
                                kc = 2 * j + kk
                                nc.tensor.matmul(
                                    lps[:, kk, :],
                                    kt[tg][base:base + 32, :, kc * 128:(kc + 1) * 128],
                                    qt[tg][base:base + 32, :, qc * 512:(qc + 1) * 512],
                                    start=True, stop=True, perf_mode=DR,
                                    tile_position=(base, 0))
                            eng = exp_sched[exp_i[0] % 128]
                            exp_i[0] += 1
                            dst = pts[:, j, :, qc * 512:(qc + 1) * 512]
                            if eng == 0:
                                nc.scalar.activation(dst, lps, AF.Exp, scale=SCALE)
                            else:
                                nc.vector.tensor_scalar(dst.bitcast(I8), lps,
                                                        A_SCH * SCALE, B_SCH,
                                                        op0=ALU.mult, op1=ALU.add)
                    pv = pvp.tile([65, 2, 512], F32, name=f"pv_{h}", tag="pv")
                    for qc in range(2):
                        for t in range(4):
                            nc.tensor.matmul(pv[:, qc, :], v65[t][:, :, h, :],
                                             pts[:, t, :, qc * 512:(qc + 1) * 512],
                                             start=(t == 0), stop=(t == 3), perf_mode=DR)
                    recip = rp.tile([1, 2, 512], F32, name=f"rc_{h}", tag="rc")
                    nc.vector.reciprocal(recip, pv[64:65, :, :])
                    rbc = rbp.tile([64, 2, 512], F32, name=f"rb_{h}", tag="rb")
                    nc.gpsimd.partition_broadcast(rbc, recip, channels=64)
                    ct, cb = h // 4, (h // 2) % 2
                    nc.vector.tensor_mul(
                        ctxp[ct][64 * (h % 2):64 * (h % 2) + 64, cb, :],
                        pv[0:64, :, :], rbc)
                    if dbg and h == 0:
                        nc.sync.dma_start(dpts_d[:, :, :, :], pts)
                        pvc = rp.tile([65, 2, 512], F32, name="dbgpv", tag="dbgpv")
                        nc.vector.tensor_copy(pvc, pv)
                    emit_outA(h)

        psum_es.close()
        ap_es.close()
        if dbg:
            for t in range(4):
                nc.sync.dma_start(dqt_d[t], qt[t])
                nc.sync.dma_start(dkt_d[t], kt[t])
                nc.sync.dma_start(dv65_d[t], v65[t])
                nc.sync.dma_start(dctx_d[t], ctxp[t])
            for st in range(8):
                nc.sync.dma_start(doa_d[st], oa[st])

        # ============ Phase C: ctx @ Wo_bot + combine ============
        with (
            tc.tile_pool(name="wobp", bufs=1) as wobp,
            tc.tile_pool(name="outp", bufs=2) as outp,
            tc.tile_pool(name="php", bufs=3, space="PSUM") as php,
        ):
            wob8 = wobp.tile([128, 4, 2, D], F8, name="wob8")
            nc.sync.dma_start(wob8, wob8_d[:, :, :, :])
            for st in range(8):
                osb = outp.tile([128, D], F32, name=f"osb{st}", tag="osb")
                for dc in range(2):
                    ps2 = php.tile([128, 512], F32, name=f"ps2_{st}_{dc}", tag="php")
                    for j in range(4):
                        nc.tensor.matmul(ps2, ctxp[j][:, :, st * 128:(st + 1) * 128],
                                         wob8[:, j, :, dc * 512:(dc + 1) * 512],
                                         start=(j == 0), stop=(j == 3), perf_mode=DR)
                    nc.vector.scalar_tensor_tensor(
                        osb[:, dc * 512:(dc + 1) * 512], ps2, OUT_S,
                        oa[st][:, dc * 512:(dc + 1) * 512],
                        op0=ALU.mult, op1=ALU.add)
                nc.sync.dma_start(out_d[st * 128:(st + 1) * 128, :], osb)

        es.close()

    nc.finalize()
    return nc


_NC_CACHE = None


def _get_nc():
    global _NC_CACHE
    if _NC_CACHE is None:
        _NC_CACHE = build_nc()
    return _NC_CACHE


def _perm():
    pi = np.empty(D, np.int64)
    for a in range(8):
        for q in range(128):
            pi[a * 128 + q] = 64 * (4 * (a // 2) + q // 32) + 32 * (a % 2) + q % 32
    return pi


def _pair4(x):
    # [1024, N] -> [128, 4, 2, N] with row 128*(2t+b)+p at [p, t, b]
    n = x.shape[1]
    return np.ascontiguousarray(x.reshape(4, 2, 128, n).transpose(2, 0, 1, 3))


def kernel(**inputs):
    global LAST_EXEC_NS
    v = np.asarray(inputs["v"], np.float32)
    k = np.asarray(inputs["k"], np.float32)
    q_in = np.asarray(inputs["q_in"], np.float32)
    mask = np.asarray(inputs["mask"], np.float32)
    wq_w = np.asarray(inputs["wq_w"], np.float32)
    wq_b = np.asarray(inputs["wq_b"], np.float32)
    wk_w = np.asarray(inputs["wk_w"], np.float32)
    wk_b = np.asarray(inputs["wk_b"], np.float32)
    wv_w = np.asarray(inputs["wv_w"], np.float32)
    wv_b = np.asarray(inputs["wv_b"], np.float32)
    wo_w = np.asarray(inputs["wo_w"], np.float32)
    wo_b = np.asarray(inputs["wo_b"], np.float32)

    pi = _perm()
    wq8 = _pair4(wq_w[:, pi].astype(NP8))
    wk8 = _pair4(wk_w[:, pi].astype(NP8))
    wv8 = _pair4(wv_w.astype(NP8))
    wob8 = _pair4((wo_w[D:] * WOB_S).astype(NP8))
    wot = np.ascontiguousarray(
        wo_w[:D].reshape(8, 128, D).transpose(1, 0, 2)).astype(NPBF)
    bor = wo_b.reshape(1, D).astype(NPBF)
    bqp = wq_b[pi].reshape(8, 128).T          # [128, 8]
    bkp = wk_b[pi].reshape(8, 128).T

    in_maps = []
    for bi in range(B):
        m = np.exp(np.float32(-1e9) * mask[bi, 0, 0, :]).astype(np.float32)
        m_st = m.reshape(8, 128).T            # [128, 8]
        cst = np.concatenate([bqp, bkp, m_st], axis=1).astype(np.float32)
        bvm = (wv_b[None, None, :] * m_st[:, :, None]).astype(NP8)  # [128, 8, 1024]
        xqT = np.ascontiguousarray(q_in[bi].T)
        in_maps.append({
            "xq8": _pair4(xqT.astype(NP8)),
            "xk8": _pair4(k[bi].T.astype(NP8)),
            "xv8": _pair4(v[bi].T.astype(NP8)),
            "wq8": wq8, "wk8": wk8, "wv8": wv8, "wob8": wob8,
            "xqb": np.ascontiguousarray(xqT.reshape(8, 128, S).transpose(1, 0, 2)).astype(NPBF),
            "wot": wot, "cst": np.ascontiguousarray(cst),
            "bvm": np.ascontiguousarray(bvm), "bor": bor,
        })

    nc = _get_nc()
    trace = os.environ.get("MHA_TRACE", "0") == "1"
    res = run_bass_kernel_spmd(nc, in_maps, core_ids=list(range(B)), trace=trace)
    LAST_EXEC_NS = res.exec_time_ns
    globals()["LAST_RES"] = res
    return np.stack([r["out"] for r in res.results], axis=0)


# revision 23
# speedup vs baseline: 1.8499x; 1.0421x over previous
"""Trainium2 Bass kernel for nn_MultiHeadAttention_73607149519012.

MHA: B=8, S=1024, D=1024, H=16 heads, depth=64, fp32 in/out.
Sharding: data-parallel over batch -- one batch element per NeuronCore (8 cores).

fp8e4 DoubleRow matmuls everywhere except the dominant q_in @ Wo_top half
(bf16).  DoubleRow packs two K-tiles per instruction at 0.5 cycles/row, so the
fp8 matmuls run ~4x faster than bf16 per MAC.  The attention path contributes
only ~2% of the output magnitude (ctx std ~0.02 vs q_in std ~1), so fp8 noise
there is diluted ~50x and the dominant half stays bf16.

Layouts (all host-prepared, no on-device transposes):
  Q/K proj : psum[dout',s] = Wq'^T x^T  with Wq' column-permuted so each psum
             chunk drains 1:1 into the DoubleRow logits layout
             qt[tile][32*(h%4)+d%32, d//32, s]  (4 heads per 128-partition tile)
  V proj   : psum[s,dv] -> v65p[kt//2][s, kt%2, head, 0:64]; column 64 holds
             mask[kpos]/32 so the PV matmul accumulates rowsum/32 in row 64
  logits   : DR lhsT=KT[32,2,128], rhs=QT[32,2,512] at tile_position (32*(h%4),0)
             -> psum[kpos128, q512];  exp has NO mask term (mask rides V rows)
  exp      : split ACT (exact Exp) / DVE (Schraudolph int8(a*x+b) bitcast fp8,
             rms ~3%, mean ~0; GPSIMD cannot read PSUM so Pool gets none)
  PV       : DR lhsT=v65p[128,2,65], rhs=pts[128,2,512] -> psum[65, 2, q512]
             (both q-chunks in one 2-bank tile); row 64 = rowsum/32;
             reciprocal -> Pool broadcast -> DVE mul gives ctx*32 in fp8
  out      : psum[s,dout] = x@Wo_top (bf16, + K=1 ones-row matmul adding bo)
             drained by ACT to bf16 oa; then psum2 = (32ctx)@(64Wo_bot) fp8 DR
             and final out = psum2/2048 + oa  (scalar_tensor_tensor)

Emission interleaves phases so the exp stream (the ACT/DVE bottleneck) starts
as early as possible: V proj first, then per-tile [Q chunks, K chunks, 4 heads].
"""

import os
from contextlib import ExitStack

import numpy as np
import ml_dtypes

import concourse.mybir as mybir
import concourse.tile as tile
from concourse import bacc
from concourse.bass_utils import run_bass_kernel_spmd

F32 = mybir.dt.float32
BF16 = mybir.dt.bfloat16
F8 = mybir.dt.float8e4
I8 = mybir.dt.int8
AF = mybir.ActivationFunctionType
ALU = mybir.AluOpType
DR = mybir.MatmulPerfMode.DoubleRow

NP8 = ml_dtypes.float8_e4m3
NPBF = ml_dtypes.bfloat16

D = 1024
S = 1024
H = 16
DEP = 64
B = 8
SCALE = 1.0 / 8.0          # 1/sqrt(DEP)
A_SCH = 8.0 / np.log(2.0)  # Schraudolph slope for 3-bit-mantissa fp8
B_SCH = 56.0 - 0.45        # bias 7*8, calibrated -0.45 to zero the mean error
CTX_S = 32.0               # ctx scaled by 32 into fp8 (ones column = 1/32)
WOB_S = 64.0               # Wo_bot scaled by 64 into fp8
OUT_S = 1.0 / (CTX_S * WOB_S)

# exp engine split ACT:DVE:(Pool unused), tunable for balance
_EW = os.environ.get("MHA_EXP_W", "83,45,0")
EXP_W = tuple(int(x) for x in _EW.split(","))

LAST_EXEC_NS = None
LAST_RES = None


def _exp_schedule():
    total = sum(EXP_W)
    counts = [w * 128 // total for w in EXP_W]
    while sum(counts) < 128:
        counts[int(np.argmax([w / (c + 1) for w, c in zip(EXP_W, counts)]))] += 1
    sched = []
    acc = [0.0, 0.0, 0.0]
    for _ in range(128):
        for i in range(3):
            acc[i] += counts[i] / 128.0
        pick = int(np.argmax(acc))
        acc[pick] -= 1.0
        sched.append(pick)
    return sched


def build_nc():
    nc = bacc.Bacc(None, target_bir_lowering=False)

    xq8_d = nc.dram_tensor("xq8", [128, 4, 2, S], F8, kind="ExternalInput")
    xk8_d = nc.dram_tensor("xk8", [128, 4, 2, S], F8, kind="ExternalInput")
    xv8_d = nc.dram_tensor("xv8", [128, 4, 2, S], F8, kind="ExternalInput")
    wq8_d = nc.dram_tensor("wq8", [128, 4, 2, D], F8, kind="ExternalInput")
    wk8_d = nc.dram_tensor("wk8", [128, 4, 2, D], F8, kind="ExternalInput")
    wv8_d = nc.dram_tensor("wv8", [128, 4, 2, D], F8, kind="ExternalInput")
    wob8_d = nc.dram_tensor("wob8", [128, 4, 2, D], F8, kind="ExternalInput")
    xqb_d = nc.dram_tensor("xqb", [128, 8, S], BF16, kind="ExternalInput")
    wot_d = nc.dram_tensor("wot", [128, 8, D], BF16, kind="ExternalInput")
    # cst cols: 0-7 bq' (permuted), 8-15 bk' (permuted), 16-23 mprime per st
    cst_d = nc.dram_tensor("cst", [128, 24], F32, kind="ExternalInput")
    bvm_d = nc.dram_tensor("bvm", [128, 8, D], F8, kind="ExternalInput")
    bor_d = nc.dram_tensor("bor", [1, D], BF16, kind="ExternalInput")
    out_d = nc.dram_tensor("out", [S, D], F32, kind="ExternalOutput")

    dbg = os.environ.get("MHA_DBG", "0") == "1"
    if dbg:
        dqt_d = nc.dram_tensor("dqt", [4, 128, 2, S], F8, kind="ExternalOutput")
        dkt_d = nc.dram_tensor("dkt", [4, 128, 2, S], F8, kind="ExternalOutput")
        dv65_d = nc.dram_tensor("dv65", [4, 128, 2, H, 65], F8, kind="ExternalOutput")
        dpts_d = nc.dram_tensor("dpts", [128, 4, 2, S], F8, kind="ExternalOutput")
        dctx_d = nc.dram_tensor("dctx", [4, 128, 2, S], F8, kind="ExternalOutput")
        doa_d = nc.dram_tensor("doa", [8, 128, D], BF16, kind="ExternalOutput")

    exp_sched = _exp_schedule()
    exp_i = [0]

    with tile.TileContext(nc) as tc:
        es = ExitStack()
        cp = es.enter_context(tc.tile_pool(name="cp", bufs=1))
        ap_es = ExitStack()
        ap = ap_es.enter_context(tc.tile_pool(name="ap", bufs=1))

        # ---- persistent tiles ----
        cst = cp.tile([128, 24], F32, name="cst")
        xqb = cp.tile([128, 8, S], BF16, name="xqb")
        wot = cp.tile([128, 8, D], BF16, name="wot")
        bvm = cp.tile([128, 8, D], F8, name="bvm")
        bor = cp.tile([1, D], BF16, name="bor")
        ones1 = cp.tile([1, 128], BF16, name="ones1")
        qt = [cp.tile([128, 2, S], F8, name=f"qt{t}", tag=f"qt{t}") for t in range(4)]
        kt = [cp.tile([128, 2, S], F8, name=f"kt{t}", tag=f"kt{t}") for t in range(4)]
        v65 = [cp.tile([128, 2, H, 65], F8, name=f"v65_{t}", tag=f"v65_{t}") for t in range(4)]
        ctxp = [cp.tile([128, 2, S], F8, name=f"ctx{t}", tag=f"ctx{t}") for t in range(4)]
        oa = [cp.tile([128, D], BF16, name=f"oa{t}", tag=f"oa{t}") for t in range(8)]

        xv8 = ap.tile([128, 4, 2, S], F8, name="xv8")
        wv8 = ap.tile([128, 4, 2, D], F8, name="wv8")
        xq8 = ap.tile([128, 4, 2, S], F8, name="xq8")
        wq8 = ap.tile([128, 4, 2, D], F8, name="wq8")
        xk8 = ap.tile([128, 4, 2, S], F8, name="xk8")
        wk8 = ap.tile([128, 4, 2, D], F8, name="wk8")

        # loads in need-order, big fp8 tensors split per k-tile-pair so the
        # first accumulation steps can start before the full tensor lands
        nc.sync.dma_start(cst, cst_d[:, :])
        nc.sync.dma_start(bvm, bvm_d[:, :, :])
        for t in range(4):
            nc.sync.dma_start(xv8[:, t], xv8_d[:, t])
            nc.sync.dma_start(wv8[:, t], wv8_d[:, t])
        for t in range(4):
            nc.sync.dma_start(xq8[:, t], xq8_d[:, t])
            nc.sync.dma_start(wq8[:, t], wq8_d[:, t])
        for t in range(4):
            nc.sync.dma_start(xk8[:, t], xk8_d[:, t])
            nc.sync.dma_start(wk8[:, t], wk8_d[:, t])
        nc.sync.dma_start(xqb, xqb_d[:, :, :])
        nc.sync.dma_start(wot, wot_d[:, :, :])
        nc.sync.dma_start(bor, bor_d[:, :])
        nc.gpsimd.memset(ones1, 1.0)

        # ones/mask columns of v65 (column 64 = mprime/32 per kpos), on Pool
        for t in range(4):
            nc.gpsimd.memset(v65[t][:, :, :, 64:65], 1.0 / CTX_S)
        for st in range(8):
            t, b = st // 2, st % 2
            nc.gpsimd.tensor_scalar_mul(
                v65[t][:, b, :, 64:65], v65[t][:, b, :, 64:65],
                cst[:, 16 + st:17 + st])

        psum_es = ExitStack()
        gp = psum_es.enter_context(tc.tile_pool(name="gp", bufs=2, space="PSUM"))

        def emit_outA(gi):
            st, dc = gi // 2, gi % 2
            psA = gp.tile([128, 512], F32, name=f"psA_{st}_{dc}", tag="gp")
            for t8 in range(8):
                nc.tensor.matmul(psA, xqb[:, t8, st * 128:(st + 1) * 128],
                                 wot[:, t8, dc * 512:(dc + 1) * 512],
                                 start=(t8 == 0), stop=False)
            nc.tensor.matmul(psA, ones1, bor[:, dc * 512:(dc + 1) * 512],
                             start=False, stop=True)
            nc.scalar.activation(oa[st][:, dc * 512:(dc + 1) * 512], psA, AF.Copy)

        # ---- V projection (own psum pool, closed before attention opens) ----
        with tc.tile_pool(name="ppV", bufs=4, space="PSUM") as ppV:
            for st in range(8):
                t, b = st // 2, st % 2
                for c in range(2):
                    ps = ppV.tile([128, 512], F32, name=f"ps_v_{st}_{c}", tag="ppV")
                    for t4 in range(4):
                        nc.tensor.matmul(
                            ps, xv8[:, t4, :, st * 128:(st + 1) * 128],
                            wv8[:, t4, :, c * 512:(c + 1) * 512],
                            start=(t4 == 0), stop=(t4 == 3), perf_mode=DR)
                    nc.vector.scalar_tensor_tensor(
                        v65[t][:, b, c * 8:(c + 1) * 8, 0:64],
                        ps.rearrange("p (h e) -> p h e", e=64),
                        cst[:, 16 + st:17 + st],
                        bvm[:, st, c * 512:(c + 1) * 512].rearrange("p (h e) -> p h e", e=64),
                        op0=ALU.mult, op1=ALU.add)

        # ---- interleaved Q/K projections + attention, by head-group tile ----
        with (
            tc.tile_pool(name="ptsp", bufs=3) as ptsp,
            tc.tile_pool(name="rp", bufs=2) as rp,
            tc.tile_pool(name="rbp", bufs=2) as rbp,
            tc.tile_pool(name="lpsp", bufs=2, space="PSUM") as lpsp,
            tc.tile_pool(name="pvp", bufs=1, space="PSUM") as pvp,
        ):
            for tg in range(4):
                for kind in ("q", "k"):
                    w_sb = wq8 if kind == "q" else wk8
                    x_sb = xq8 if kind == "q" else xk8
                    dst = qt if kind == "q" else kt
                    bcol = 0 if kind == "q" else 8
                    for a in (2 * tg, 2 * tg + 1):
                        for sc in range(2):
                            ps = gp.tile([128, 512], F32, name=f"ps_{kind}_{a}_{sc}", tag="gp")
                            for t in range(4):
                                nc.tensor.matmul(
                                    ps, w_sb[:, t, :, a * 128:(a + 1) * 128],
                                    x_sb[:, t, :, sc * 512:(sc + 1) * 512],
                                    start=(t == 0), stop=(t == 3), perf_mode=DR)
                            nc.scalar.activation(
                                dst[a // 2][:, a % 2, sc * 512:(sc + 1) * 512], ps,
                                AF.Identity, bias=cst[:, bcol + a:bcol + a + 1])
                for h in range(4 * tg, 4 * tg + 4):
                    jj = h % 4
                    base = 32 * jj
                    pts = ptsp.tile([128, 4, 2, S], F8, name=f"pts{h}", tag="pts")
                    for qc in range(2):
                        for j in range(4):
                            lps = lpsp.tile([128, 2, 512], F32, name=f"lps_{h}_{qc}_{j}", tag="lps")
                            for kk in range(2):
                                kc = 2 * j + kk
                                nc.tensor.matmul(
                                    lps[:, kk, :],
                                    kt[tg][base:base + 32, :, kc * 128:(kc + 1) * 128],
                                    qt[tg][base:base + 32, :, qc * 512:(qc + 1) * 512],
                                    start=True, stop=True, perf_mode=DR,
                                    tile_position=(base, 0))
                            eng = exp_sched[exp_i[0] % 128]
                            exp_i[0] += 1
                            dst = pts[:, j, :, qc * 512:(qc + 1) * 512]
                            if eng == 0:
                                nc.scalar.activation(dst, lps, AF.Exp, scale=SCALE)
                            else:
                                nc.vector.tensor_scalar(dst.bitcast(I8), lps,
                                                        A_SCH * SCALE, B_SCH,
                                                        op0=ALU.mult, op1=ALU.add)
                    pv = pvp.tile([65, 2, 512], F32, name=f"pv_{h}", tag="pv")
                    for qc in range(2):
                        for t in range(4):
                            nc.tensor.matmul(pv[:, qc, :], v65[t][:, :, h, :],
                                             pts[:, t, :, qc * 512:(qc + 1) * 512],
                                             start=(t == 0), stop=(t == 3), perf_mode=DR)
                    recip = rp.tile([1, 2, 512], F32, name=f"rc_{h}", tag="rc")
                    nc.vector.reciprocal(recip, pv[64:65, :, :])
                    rbc = rbp.tile([64, 2, 512], F32, name=f"rb_{h}", tag="rb")
                    nc.gpsimd.partition_broadcast(rbc, recip, channels=64)
                    ct, cb = h // 4, (h // 2) % 2
                    nc.vector.tensor_mul(
                        ctxp[ct][64 * (h % 2):64 * (h % 2) + 64, cb, :],
                        pv[0:64, :, :], rbc)
                    if dbg and h == 0:
                        nc.sync.dma_start(dpts_d[:, :, :, :], pts)
                        pvc = rp.tile([65, 2, 512], F32, name="dbgpv", tag="dbgpv")
                        nc.vector.tensor_copy(pvc, pv)
                    emit_outA(h)

        psum_es.close()
        ap_es.close()
        if dbg:
            for t in range(4):
                nc.sync.dma_start(dqt_d[t], qt[t])
                nc.sync.dma_start(dkt_d[t], kt[t])
                nc.sync.dma_start(dv65_d[t], v65[t])
                nc.sync.dma_start(dctx_d[t], ctxp[t])
            for st in range(8):
                nc.sync.dma_start(doa_d[st], oa[st])

        # ============ Phase C: ctx @ Wo_bot + combine ============
        with (
            tc.tile_pool(name="wobp", bufs=1) as wobp,
            tc.tile_pool(name="outp", bufs=4) as outp,
            tc.tile_pool(name="php", bufs=4, space="PSUM") as php,
        ):
            wob8 = wobp.tile([128, 4, 2, D], F8, name="wob8")
            nc.sync.dma_start(wob8, wob8_d[:, :, :, :])
            for st in range(8):
                osb = outp.tile([128, D], F32, name=f"osb{st}", tag="osb")
                for dc in range(2):
                    ps2 = php.tile([128, 512], F32, name=f"ps2_{st}_{dc}", tag="php")
                    for j in range(4):
                        nc.tensor.matmul(ps2, ctxp[j][:, :, st * 128:(st + 1) * 128],
                                         wob8[:, j, :, dc * 512:(dc + 1) * 512],
                                         start=(j == 0), stop=(j == 3), perf_mode=DR)
                    nc.vector.scalar_tensor_tensor(
                        osb[:, dc * 512:(dc + 1) * 512], ps2, OUT_S,
                        oa[st][:, dc * 512:(dc + 1) * 512],
                        op0=ALU.mult, op1=ALU.add)
                nc.sync.dma_start(out_d[st * 128:(st + 1) * 128, :], osb)

        es.close()

    nc.finalize()
    return nc


_NC_CACHE = None


def _get_nc():
    global _NC_CACHE
    if _NC_CACHE is None:
        _NC_CACHE = build_nc()
    return _NC_CACHE


def _perm():
    pi = np.empty(D, np.int64)
    for a in range(8):
        for q in range(128):
            pi[a * 128 + q] = 64 * (4 * (a // 2) + q // 32) + 32 * (a % 2) + q % 32
    return pi


def _pair4(x):
    # [1024, N] -> [128, 4, 2, N] with row 128*(2t+b)+p at [p, t, b]
    n = x.shape[1]
    return np.ascontiguousarray(x.reshape(4, 2, 128, n).transpose(2, 0, 1, 3))


def kernel(**inputs):
    global LAST_EXEC_NS
    v = np.asarray(inputs["v"], np.float32)
    k = np.asarray(inputs["k"], np.float32)
    q_in = np.asarray(inputs["q_in"], np.float32)
    mask = np.asarray(inputs["mask"], np.float32)
    wq_w = np.asarray(inputs["wq_w"], np.float32)
    wq_b = np.asarray(inputs["wq_b"], np.float32)
    wk_w = np.asarray(inputs["wk_w"], np.float32)
    wk_b = np.asarray(inputs["wk_b"], np.float32)
    wv_w = np.asarray(inputs["wv_w"], np.float32)
    wv_b = np.asarray(inputs["wv_b"], np.float32)
    wo_w = np.asarray(inputs["wo_w"], np.float32)
    wo_b = np.asarray(inputs["wo_b"], np.float32)

    pi = _perm()
    wq8 = _pair4(wq_w[:, pi].astype(NP8))
    wk8 = _pair4(wk_w[:, pi].astype(NP8))
    wv8 = _pair4(wv_w.astype(NP8))
    wob8 = _pair4((wo_w[D:] * WOB_S).astype(NP8))
    wot = np.ascontiguousarray(
        wo_w[:D].reshape(8, 128, D).transpose(1, 0, 2)).astype(NPBF)
    bor = wo_b.reshape(1, D).astype(NPBF)
    bqp = wq_b[pi].reshape(8, 128).T          # [128, 8]
    bkp = wk_b[pi].reshape(8, 128).T

    in_maps = []
    for bi in range(B):
        m = np.exp(np.float32(-1e9) * mask[bi, 0, 0, :]).astype(np.float32)
        m_st = m.reshape(8, 128).T            # [128, 8]
        cst = np.concatenate([bqp, bkp, m_st], axis=1).astype(np.float32)
        bvm = (wv_b[None, None, :] * m_st[:, :, None]).astype(NP8)  # [128, 8, 1024]
        xqT = np.ascontiguousarray(q_in[bi].T)
        in_maps.append({
            "xq8": _pair4(xqT.astype(NP8)),
            "xk8": _pair4(k[bi].T.astype(NP8)),
            "xv8": _pair4(v[bi].T.astype(NP8)),
            "wq8": wq8, "wk8": wk8, "wv8": wv8, "wob8": wob8,
            "xqb": np.ascontiguousarray(xqT.reshape(8, 128, S).transpose(1, 0, 2)).astype(NPBF),
            "wot": wot, "cst": np.ascontiguousarray(cst),
            "bvm": np.ascontiguousarray(bvm), "bor": bor,
        })

    nc = _get_nc()
    trace = os.environ.get("MHA_TRACE", "0") == "1"
    res = run_bass_kernel_spmd(nc, in_maps, core_ids=list(range(B)), trace=trace)
    LAST_EXEC_NS = res.exec_time_ns
    globals()["LAST_RES"] = res
    return np.stack([r["out"] for r in res.results], axis=0)
